# revision 1
# baseline (speedup 1.0000x reference)
"""BertSelfAttention (disentangled seg-bias variant) on 8 Trainium2 NeuronCores.

Sharding: tensor-parallel over heads (2 heads per core), data-parallel batch
handled inside each core (both batches per core, rel_pos tiles shared).

Math per (b, h):
  q = hs @ Wq.T + bq ; k = hs @ Wk.T ; v = hs @ Wv.T + bv
  k' = scale*k + seg_rep          (seg folded into K-projection PSUM via a
                                   rank-2 matmul: seg_rep = t0*(1-s) + t1*s)
  scoresT[j,i] = sum_c k'[c,j] q[c,i]            (j on partitions)
  r1[j] = b_q_s . seg_rep[j]  -> per-partition bias of the exp
  probsT = exp(scoresT + relT + r1[j])           (no max-subtraction; |s|<~10)
  ctxT[c,i] = sum_j v[j,c] probsT[j,i] ; denom via ones-columns in the same MM
  out = ctxT / denom
"""

import os
import numpy as np
from contextlib import ExitStack

import concourse.bass as bass
import concourse.bacc as bacc
import concourse.mybir as mybir
import concourse.tile as tile
from concourse.bass_utils import run_bass_kernel_spmd
from concourse.masks import make_identity

B, S, D, H = 2, 2048, 1024, 16
DH = D // H                      # 64
N_CORES = 8
HPC = H // N_CORES               # heads per core = 2
NKC = D // 128                   # contraction chunks = 8
NPT = S // 512                   # 512-wide position tiles = 4
NJT = S // 128                   # 128-wide j tiles = 16
NIB = S // 1024                  # 1024-wide i blocks = 2
SCALE = 1.0 / np.sqrt(DH)        # 0.125, exact in fp16

F32 = mybir.dt.float32
F16 = mybir.dt.float16

# j-tiles (0..15) whose rel_pos tile is injected into PSUM by the tensor
# engine (identity matmul) instead of added on the vector engine.  Used to
# balance PE vs DVE load.
INJECT_JTS = frozenset()
# j-tiles whose rel tile goes through exp-factorization:
# exp(qk+rel) = exp(qk) * exp(rel); exp(rel) shared across batches (ScalarE),
# multiply on VectorE at 2x fp16 rate instead of the 1x fp32-PSUM add.
FACTORIZE_JTS = frozenset({1, 3, 5, 7, 9, 11, 13, 15})


def emit_body(nc, tc, ctx, pools, aps, use_mask, inject_jts, factorize_jts,
              opts=None):
    opts = opts or {}
    (const, hspool, qpool, kpool, vtpool, vnpool, relpool, addpool, probpool,
     pspool, pvpool, denpool, rcpbpool, ctxpool, scrpool) = pools
    hsT, wT, relT, seg2, segc, stab, stabf, bqs, bqc, bvc, maskT, out = aps

    w_sb = const.tile([128, 3, NKC, 128], F16, tag="w_sb")
    for p in range(3):
        nc.sync.dma_start(out=w_sb[:, p], in_=wT[p].rearrange("k d c -> d k c"))
    # fold softmax scale into Wk (0.125 is exact in fp16)
    nc.vector.tensor_scalar_mul(w_sb[:, 1], w_sb[:, 1], SCALE)

    stab_sb = const.tile([2, 128], F16, tag="stab_sb")
    nc.sync.dma_start(out=stab_sb, in_=stab)
    seg2_sb = const.tile([2, B * S], F16, tag="seg2_sb")
    nc.sync.dma_start(out=seg2_sb, in_=seg2.rearrange("b r s -> r b s"))

    t0f = const.tile([1, 128], F32, tag="t0f")
    t1f = const.tile([1, 128], F32, tag="t1f")
    nc.sync.dma_start(out=t0f, in_=stabf[0:1])
    nc.sync.dma_start(out=t1f, in_=stabf[1:2])
    bqs_sb = const.tile([1, 128], F32, tag="bqs_sb")
    nc.sync.dma_start(out=bqs_sb, in_=bqs)
    bqc_sb = const.tile([128, 1], F32, tag="bqc_sb")
    nc.sync.dma_start(out=bqc_sb, in_=bqc)
    bvc_sb = const.tile([128, 1], F32, tag="bvc_sb")
    nc.sync.dma_start(out=bvc_sb, in_=bvc)
    segc_sb = const.tile([128, B * NJT], F32, tag="segc_sb")
    nc.sync.dma_start(out=segc_sb, in_=segc.rearrange("b p t -> p b t"))

    ident = const.tile([128, 128], F16, tag="ident")
    make_identity(nc, ident)

    # --- r1 (b_q_s . seg_rep) per-partition bias columns -------------------
    # gamma0_h = sum_{c in head h} bqs[c]*t0[c]; gamma1_h likewise with t1.
    # r1[j] = gamma0_h + (gamma1_h - gamma0_h) * s_j
    prod = const.tile([1, 128], F32, tag="prod")
    g_row = const.tile([1, 4], F32, tag="g_row")   # [g0_h0, g0_h1, g1_h0, g1_h1]
    b_row = const.tile([1, 4], F32, tag="b_row")   # [d_h0, d_h1, g0_h0, g0_h1]
    ones1 = const.tile([1, 128], F32, tag="ones1")
    nc.vector.memset(ones1, 1.0)
    nc.vector.tensor_mul(prod, bqs_sb, t0f)
    nc.vector.tensor_reduce(g_row[0:1, 0:1], prod[0:1, 0:64],
                            axis=mybir.AxisListType.X, op=mybir.AluOpType.add)
    nc.vector.tensor_reduce(g_row[0:1, 1:2], prod[0:1, 64:128],
                            axis=mybir.AxisListType.X, op=mybir.AluOpType.add)
    nc.vector.tensor_mul(prod, bqs_sb, t1f)
    nc.vector.tensor_reduce(g_row[0:1, 2:3], prod[0:1, 0:64],
                            axis=mybir.AxisListType.X, op=mybir.AluOpType.add)
    nc.vector.tensor_reduce(g_row[0:1, 3:4], prod[0:1, 64:128],
                            axis=mybir.AxisListType.X, op=mybir.AluOpType.add)
    # delta_h = g1_h - g0_h in slots 0:2, g0_h in slots 2:4
    nc.vector.tensor_sub(b_row[0:1, 0:2], g_row[0:1, 2:4], g_row[0:1, 0:2])
    nc.vector.tensor_copy(b_row[0:1, 2:4], g_row[0:1, 0:2])
    # broadcast the 4 values to all 128 partitions via a K=1 matmul
    psB = pspool.tile([128, 4], F32, tag="ps_s")
    nc.tensor.matmul(psB, lhsT=ones1, rhs=b_row, start=True, stop=True)
    bc4 = const.tile([128, 4], F32, tag="bc4")
    nc.vector.tensor_copy(bc4, psB)
    # r1 columns, [128, (b*2+hl)*16 + jt]
    r1c = const.tile([128, B * HPC * NJT], F32, tag="r1c")
    for b in range(B):
        for hl in range(HPC):
            nc.vector.tensor_scalar(
                out=r1c[:, (b * HPC + hl) * NJT:(b * HPC + hl + 1) * NJT],
                in0=segc_sb[:, b * NJT:(b + 1) * NJT],
                scalar1=bc4[:, hl:hl + 1],
                scalar2=bc4[:, 2 + hl:2 + hl + 1],
                op0=mybir.AluOpType.mult,
                op1=mybir.AluOpType.add,
            )

    # --- Stage A: projections -> qT, k'T, v_nat ---------------------------
    qT, kT, vn = [None] * B, [None] * B, [None] * B
    if opts.get("no_proj"):
        static_qk = const.tile([128, S], F16, tag="static_qk")
        nc.vector.memset(static_qk, 0.01)
        static_vn = const.tile([128, NJT, DH + 4], F16, tag="static_vn")
        nc.vector.memset(static_vn, 0.01)
        for _b in range(B):
            qT[_b] = static_qk
            kT[_b] = static_qk
            vn[_b] = [static_vn, static_vn]

    def emit_proj(b):
        if opts.get("no_proj"):
            return
        hsb = hspool.tile([128, NKC, S], F16, tag="hsb", name=f"hsb{b}")
        for kk in range(NKC):
            nc.sync.dma_start(out=hsb[:, kk], in_=hsT[b, kk])
        qT_b = qpool.tile([128, S], F16, tag="qT", name=f"qT{b}")
        kT_b = kpool.tile([128, S], F16, tag="kT", name=f"kT{b}")
        vTt = vtpool.tile([128, S], F16, tag="vTt", name=f"vTt{b}")
        for pt in range(NPT if not opts.get("no_proj") else 0):
            sl = bass.ds(pt * 512, 512)
            psQ = pspool.tile([128, 512], F32, tag="ps_s", name="psQ")
            for kk in range(NKC):
                nc.tensor.matmul(psQ, lhsT=w_sb[:, 0, kk], rhs=hsb[:, kk, sl],
                                 start=(kk == 0), stop=(kk == NKC - 1))
            if opts.get("qv_dve"):
                nc.vector.tensor_scalar_add(qT_b[:, sl], psQ, bqc_sb)
            else:
                nc.scalar.add(qT_b[:, sl], psQ, bqc_sb)
            psK = pspool.tile([128, 512], F32, tag="ps_s", name="psK")
            for kk in range(NKC):
                nc.tensor.matmul(psK, lhsT=w_sb[:, 1, kk], rhs=hsb[:, kk, sl],
                                 start=(kk == 0), stop=False)
            nc.tensor.matmul(psK, lhsT=stab_sb,
                             rhs=seg2_sb[:, bass.ds(b * S + pt * 512, 512)],
                             start=False, stop=True)
            nc.vector.tensor_copy(kT_b[:, sl], psK)
            psV = pspool.tile([128, 512], F32, tag="ps_s", name="psV")
            for kk in range(NKC):
                nc.tensor.matmul(psV, lhsT=w_sb[:, 2, kk], rhs=hsb[:, kk, sl],
                                 start=(kk == 0), stop=(kk == NKC - 1))
            if opts.get("qv_dve"):
                nc.vector.tensor_scalar_add(vTt[:, sl], psV, bvc_sb)
            else:
                nc.scalar.add(vTt[:, sl], psV, bvc_sb)
        # v_nat per head: [j, (v cols 0:64 | ones 64:68)] via PE transpose
        vn_b = [vnpool.tile([128, NJT, DH + 4], F16, tag=f"vn{hl}",
                            name=f"vn_b{b}h{hl}") for hl in range(HPC)]
        for jt in range(NJT if not opts.get("no_proj") else 0):
            pst = pspool.tile([128, 128], F16, tag="ps_s", name="pst")
            nc.tensor.transpose(pst, vTt[:, bass.ds(jt * 128, 128)], ident)
            for hl in range(HPC):
                nc.vector.tensor_copy(vn_b[hl][:, jt, bass.ds(0, DH)],
                                      pst[:, bass.ds(hl * DH, DH)])
                nc.gpsimd.memset(vn_b[hl][:, jt, bass.ds(DH, 4)], 1.0)
        qT[b], kT[b], vn[b] = qT_b, kT_b, vn_b

    # --- Stage B ----------------------------------------------------------
    rel = {}
    relx = {}
    static_prob = None
    if opts.get("no_prob"):
        static_prob = const.tile([128, 1024], F16, tag="static_prob")
        nc.vector.memset(static_prob, 0.001)
    static_score = None
    if opts.get("no_qk"):
        static_score = const.tile([128, 1024], F32, tag="static_score")
        nc.vector.memset(static_score, 0.01)

    def emit_rel(ib):
        ibs = bass.ds(ib * 1024, 1024)
        n_fact = len([j for j in range(NJT) if j in factorize_jts])
        if use_mask:
            n_fact = 0
        relb = 2 * (NJT - n_fact) + 2
        relxb = 2 * n_fact + 2
        for jt in range(NJT):
            fact = (jt in factorize_jts) and not use_mask
            for hl in range(HPC):
                tag = "relf" if fact else "rel"
                r = relpool.tile([128, 1024], F16, tag=tag, name=tag,
                                 bufs=(3 if fact else relb))
                nc.sync.dma_start(out=r, in_=relT[hl, bass.ds(jt * 128, 128), ibs])
                if fact:
                    rx = relpool.tile([128, 1024], F16, tag="relx", name="relx",
                                      bufs=relxb)
                    nc.scalar.activation(rx, r, mybir.ActivationFunctionType.Exp)
                    relx[jt, hl] = rx
                else:
                    rel[jt, hl] = r

    def emit_attn(ib, b):
        pv = [pvpool.tile([DH + 4, 1024], F32, tag="pv", name=f"pv{_hl}")
              for _hl in range(HPC)]
        ibs = bass.ds(ib * 1024, 1024)
        for jt in range(NJT):
            if use_mask:
                msk = addpool.tile([128, 1024], F16, tag="msk")
                nc.sync.dma_start(
                    out=msk, in_=maskT[b, bass.ds(jt * 128, 128), ibs])
            inject = (jt in inject_jts) and not use_mask
            fact = (jt in factorize_jts) and not use_mask
            if opts.get("no_qk"):
                psS_all = [static_score, static_score]
            else:
                psS_all = [pspool.tile([128, 1024], F32, tag="ps_s",
                                       name=f"psS{_hl}") for _hl in range(HPC)]
            for i2 in range(2 if not opts.get("no_qk") else 0):
                osl = bass.ds(i2 * 512, 512)
                qsl = bass.ds(ib * 1024 + i2 * 512, 512)
                if inject:
                    for hl in range(HPC):
                        nc.tensor.matmul(psS_all[hl][:, osl], lhsT=ident,
                                         rhs=rel[jt, hl][:, osl],
                                         start=True, stop=False)
                # the two heads' K=64 matmuls are adjacent -> PE row-group
                # packing runs them concurrently (base partitions 0 and 64)
                for hl in range(HPC):
                    hs_ = bass.ds(hl * DH, DH)
                    nc.tensor.matmul(psS_all[hl][:, osl],
                                     lhsT=kT[b][hs_, bass.ds(jt * 128, 128)],
                                     rhs=qT[b][hs_, qsl],
                                     start=not inject, stop=True)
            for hl in range(HPC):
                hs_ = bass.ds(hl * DH, DH)
                psS = psS_all[hl]
                col = (b * HPC + hl) * NJT + jt
                bias_ap = r1c[:, col:col + 1]
                if opts.get("no_prob"):
                    prob = static_prob
                    if False:
                        pass
                    for i2 in range(2 if not opts.get("no_pv") else 0):
                        nc.tensor.matmul(
                            pv[hl][:, bass.ds(i2 * 512, 512)],
                            lhsT=vn[b][hl][:, jt, :],
                            rhs=prob[:, bass.ds(i2 * 512, 512)],
                            start=(jt == 0), stop=(jt == NJT - 1))
                    continue
                prob = probpool.tile([128, 1024], F16, tag="prob")
                if inject:
                    nc.scalar.activation(prob, psS,
                                         mybir.ActivationFunctionType.Exp,
                                         bias=bias_ap, scale=1.0)
                elif fact:
                    eqk = probpool.tile([128, 1024], F16, tag="eqk")
                    nc.scalar.activation(eqk, psS,
                                         mybir.ActivationFunctionType.Exp,
                                         bias=bias_ap, scale=1.0)
                    nc.vector.tensor_mul(prob, eqk, relx[jt, hl])
                else:
                    padd = addpool.tile([128, 1024], F16, tag="padd")
                    nc.vector.tensor_add(padd, psS, rel[jt, hl])
                    if use_mask:
                        padd2 = addpool.tile([128, 1024], F16, tag="padd2")
                        nc.vector.tensor_add(padd2, padd, msk)
                        padd = padd2
                    nc.scalar.activation(prob, padd,
                                         mybir.ActivationFunctionType.Exp,
                                         bias=bias_ap, scale=1.0)
                for i2 in range(2 if not opts.get("no_pv") else 0):
                    nc.tensor.matmul(
                        pv[hl][:, bass.ds(i2 * 512, 512)],
                        lhsT=vn[b][hl][:, jt, :],
                        rhs=prob[:, bass.ds(i2 * 512, 512)],
                        start=(jt == 0), stop=(jt == NJT - 1))
        return pv

    def emit_fin(ib, b, pv):
        if opts.get("no_fin"):
            return
        ibs = bass.ds(ib * 1024, 1024)
        for hl in range(HPC):
            # evacuate PSUM accumulator promptly so the slot frees up
            pvs = ctxpool.tile([DH + 1, 1024], F32, tag="pvs", name="pvs")
            if opts.get("evac_dve"):
                nc.vector.tensor_copy(pvs, pv[hl][0:DH + 1, :])
            else:
                nc.scalar.copy(pvs, pv[hl][0:DH + 1, :])
            den_dram = scrpool.tile([1, 1024], F32, tag="den_dram")
            rcp_dram = scrpool.tile([1, 1024], F32, tag="rcp_dram")
            nc.sync.dma_start(out=den_dram, in_=pvs[DH:DH + 1, :])
            den_t = denpool.tile([128, 8], F32, tag="den_t")
            nc.sync.dma_start(
                out=den_t,
                in_=bass.AP(den_dram.tensor, den_dram.offset, [[1, 128], [128, 8]]))
            rcp_t = denpool.tile([128, 8], F32, tag="rcp_t")
            nc.vector.reciprocal(rcp_t, den_t)
            nc.sync.dma_start(
                out=bass.AP(rcp_dram.tensor, rcp_dram.offset, [[1, 128], [128, 8]]),
                in_=rcp_t)
            rcpb = rcpbpool.tile([DH, 1024], F32, tag="rcpb")
            nc.sync.dma_start(
                out=rcpb,
                in_=bass.AP(rcp_dram.tensor, rcp_dram.offset, [[0, DH], [1, 1024]]))
            ctxt = ctxpool.tile([DH, 1024], F32, tag="ctxt")
            nc.vector.tensor_mul(ctxt, pvs[0:DH, :], rcpb)
            nc.sync.dma_start(
                out=out[b, bass.ds(hl * DH, DH), ibs], in_=ctxt)

    # emission order interleaves batch-1 projections under stage B(ib0, b0)
    emit_proj(0)
    emit_rel(0)
    pv00 = emit_attn(0, 0)
    emit_proj(1)
    emit_fin(0, 0, pv00)
    pv01 = emit_attn(0, 1)
    emit_fin(0, 1, pv01)
    emit_rel(1)
    pv10 = emit_attn(1, 0)
    emit_fin(1, 0, pv10)
    pv11 = emit_attn(1, 1)
    emit_fin(1, 1, pv11)


def build_nc(use_mask=False, n_reps=1, inject_jts=INJECT_JTS,
             factorize_jts=None, opts=None):
    nc = bacc.Bacc("TRN2", target_bir_lowering=False, debug=False,
                   num_devices=N_CORES)
    hsT = nc.declare_dram_parameter("hsT", [B, NKC, 128, S], F16, isOutput=False).ap()
    wT = nc.declare_dram_parameter("wT", [3, NKC, 128, 128], F16, isOutput=False).ap()
    relT = nc.declare_dram_parameter("relT", [HPC, S, S], F16, isOutput=False).ap()
    seg2 = nc.declare_dram_parameter("seg2", [B, 2, S], F16, isOutput=False).ap()
    segc = nc.declare_dram_parameter("segc", [B, 128, NJT], F32, isOutput=False).ap()
    stab = nc.declare_dram_parameter("stab", [2, 128], F16, isOutput=False).ap()
    stabf = nc.declare_dram_parameter("stabf", [2, 128], F32, isOutput=False).ap()
    bqs = nc.declare_dram_parameter("bqs", [1, 128], F32, isOutput=False).ap()
    bqc = nc.declare_dram_parameter("bqc", [128, 1], F32, isOutput=False).ap()
    bvc = nc.declare_dram_parameter("bvc", [128, 1], F32, isOutput=False).ap()
    maskT = (nc.declare_dram_parameter("maskT", [B, S, S], F16, isOutput=False).ap()
             if use_mask else None)
    out = nc.declare_dram_parameter("out", [B, 128, S], F32, isOutput=True).ap()
    aps = (hsT, wT, relT, seg2, segc, stab, stabf, bqs, bqc, bvc, maskT, out)

    with tile.TileContext(nc) as tc, ExitStack() as ctx:
        pools = (
            ctx.enter_context(tc.tile_pool(name="const", bufs=1)),
            ctx.enter_context(tc.tile_pool(name="hspool", bufs=1)),
            ctx.enter_context(tc.tile_pool(name="qpool", bufs=B)),
            ctx.enter_context(tc.tile_pool(name="kpool", bufs=B)),
            ctx.enter_context(tc.tile_pool(name="vtpool", bufs=2)),
            ctx.enter_context(tc.tile_pool(name="vnpool", bufs=B)),
            ctx.enter_context(tc.tile_pool(name="relpool", bufs=3)),
            ctx.enter_context(tc.tile_pool(name="addpool", bufs=3)),
            ctx.enter_context(tc.tile_pool(name="probpool", bufs=4)),
            ctx.enter_context(tc.tile_pool(name="pspool", bufs=2, space="PSUM")),
            ctx.enter_context(tc.tile_pool(name="pvpool", bufs=2, space="PSUM")),
            ctx.enter_context(tc.tile_pool(name="denpool", bufs=2)),
            ctx.enter_context(tc.tile_pool(name="rcpbpool", bufs=2)),
            ctx.enter_context(tc.tile_pool(name="ctxpool", bufs=2)),
            ctx.enter_context(tc.tile_pool(name="scrpool", bufs=2, space="DRAM")),
        )
        if factorize_jts is None:
            factorize_jts = FACTORIZE_JTS
        if n_reps == 1:
            emit_body(nc, tc, ctx, pools, aps, use_mask, inject_jts,
                      factorize_jts, opts)
        else:
            hint = (mybir.EngineType.PE, mybir.EngineType.DVE,
                    mybir.EngineType.Activation, mybir.EngineType.SP,
                    mybir.EngineType.Pool)
            with tc.For_i(0, n_reps, 1, hint_engines=hint):
                emit_body(nc, tc, ctx, pools, aps, use_mask, inject_jts,
                          factorize_jts, opts)
    nc.compile()
    return nc


# ---------------------------------------------------------------------------
# host side
# ---------------------------------------------------------------------------

def prep_in_maps(hidden_states, attention_mask, rel_pos, seg_ids,
                 Wq, bq, Wk, Wv, bv, seg_table, b_q_s, use_mask):
    hs = np.asarray(hidden_states, np.float32)
    hsT = np.ascontiguousarray(hs.transpose(0, 2, 1)).astype(np.float16)
    hsT = hsT.reshape(B, NKC, 128, S)
    seg = np.asarray(seg_ids).astype(np.float32)          # [B, S]
    seg2 = np.stack([1.0 - seg, seg], axis=1).astype(np.float16)  # [B,2,S]
    segc = np.ascontiguousarray(
        seg.reshape(B, NJT, 128).transpose(0, 2, 1)).astype(np.float32)
    rel = np.asarray(rel_pos, np.float32)[0]              # [H, S, S]
    relT_all = np.ascontiguousarray(rel.transpose(0, 2, 1)).astype(np.float16)
    Wq = np.asarray(Wq, np.float32); Wk = np.asarray(Wk, np.float32)
    Wv = np.asarray(Wv, np.float32)
    seg_table = np.asarray(seg_table, np.float32)
    b_q_s = np.asarray(b_q_s, np.float32)
    bq = np.asarray(bq, np.float32); bv = np.asarray(bv, np.float32)
    if use_mask:
        maskT_all = np.ascontiguousarray(
            np.asarray(attention_mask, np.float32)[:, 0].transpose(0, 2, 1)
        ).astype(np.float16)

    in_maps = []
    for c in range(N_CORES):
        hc = slice(c * HPC * DH, (c + 1) * HPC * DH)      # 128 head-columns
        wT = np.stack([
            np.ascontiguousarray(Wq[hc].T),
            np.ascontiguousarray(Wk[hc].T),
            np.ascontiguousarray(Wv[hc].T),
        ]).astype(np.float16).reshape(3, NKC, 128, 128)
        m = {
            "hsT": hsT,
            "wT": wT,
            "relT": relT_all[c * HPC:(c + 1) * HPC],
            "seg2": seg2,
            "segc": segc,
            "stab": seg_table[:, hc].astype(np.float16),
            "stabf": seg_table[:, hc].astype(np.float32),
            "bqs": b_q_s[0, c * HPC:(c + 1) * HPC, 0].reshape(1, 128).astype(np.float32),
            "bqc": bq[hc].reshape(128, 1).astype(np.float32),
            "bvc": bv[hc].reshape(128, 1).astype(np.float32),
        }
        if use_mask:
            m["maskT"] = maskT_all
        in_maps.append(m)
    return in_maps


def assemble_output(results):
    out = np.empty((B, S, D), np.float32)
    for c in range(N_CORES):
        ctxT = results[c]["out"]                          # [B, 128, S]
        hc = slice(c * HPC * DH, (c + 1) * HPC * DH)
        out[:, :, hc] = ctxT.transpose(0, 2, 1)
    return out


_CACHED = {}


def kernel(**inputs):
    use_mask = bool(np.any(np.asarray(inputs["attention_mask"])))
    key = ("nc", use_mask)
    if key not in _CACHED:
        _CACHED[key] = build_nc(use_mask=use_mask)
    nc = _CACHED[key]
    in_maps = prep_in_maps(use_mask=use_mask, **inputs)
    res = run_bass_kernel_spmd(nc, in_maps, list(range(N_CORES)))
    return assemble_output(res.results)



# revision 16
# speedup vs baseline: 1.0967x; 1.0967x over previous
"""BertSelfAttention (disentangled seg-bias variant) on 8 Trainium2 NeuronCores.

Sharding: tensor-parallel over heads (2 heads per core), data-parallel batch
handled inside each core (both batches per core, rel_pos tiles shared).

Math per (b, h):
  q = hs @ Wq.T + bq ; k = hs @ Wk.T ; v = hs @ Wv.T + bv
  k' = scale*k + seg_rep          (seg folded into K-projection PSUM via a
                                   rank-2 matmul: seg_rep = t0*(1-s) + t1*s)
  scoresT[j,i] = sum_c k'[c,j] q[c,i]            (j on partitions)
  r1[j] = b_q_s . seg_rep[j]  -> per-partition bias of the exp
  probsT = exp(scoresT + relT + r1[j])           (no max-subtraction; |s|<~10)
  ctxT[c,i] = sum_j v[j,c] probsT[j,i] ; denom via ones-columns in the same MM
  out = ctxT / denom
"""

import os
import numpy as np
from contextlib import ExitStack

import concourse.bass as bass
import concourse.bacc as bacc
import concourse.mybir as mybir
import concourse.tile as tile
from concourse.bass_utils import run_bass_kernel_spmd
from concourse.masks import make_identity

B, S, D, H = 2, 2048, 1024, 16
DH = D // H                      # 64
N_CORES = 8
HPC = H // N_CORES               # heads per core = 2
NKC = D // 128                   # contraction chunks = 8
NPT = S // 512                   # 512-wide position tiles = 4
NJT = S // 128                   # 128-wide j tiles = 16
NIB = S // 1024                  # 1024-wide i blocks = 2
SCALE = 1.0 / np.sqrt(DH)        # 0.125, exact in fp16

F32 = mybir.dt.float32
F16 = mybir.dt.float16

# j-tiles (0..15) whose rel_pos tile is injected into PSUM by the tensor
# engine (identity matmul) instead of added on the vector engine.  Used to
# balance PE vs DVE load.
INJECT_JTS = frozenset()
# j-tiles whose rel tile goes through exp-factorization:
# exp(qk+rel) = exp(qk) * exp(rel); exp(rel) shared across batches (ScalarE),
# multiply on VectorE at 2x fp16 rate instead of the 1x fp32-PSUM add.
FACTORIZE_JTS = frozenset({1, 3, 5, 7, 9, 11, 13, 15})


def emit_body(nc, tc, ctx, pools, aps, use_mask, inject_jts, factorize_jts,
              opts=None):
    opts = opts or {}
    (const, hspool, qpool, kpool, vtpool, vnpool, relpool, addpool, probpool,
     pspool, pvpool, denpool, rcpbpool, ctxpool, scrpool) = pools
    hsT, wT, relT, seg2, segc, stab, stabf, bqs, bqc, bvc, maskT, out = aps

    w_sb = const.tile([128, 3, NKC, 128], F16, tag="w_sb")
    for p in range(3):
        nc.sync.dma_start(out=w_sb[:, p], in_=wT[p].rearrange("k d c -> d k c"))

    stab_sb = const.tile([2, 128], F16, tag="stab_sb")
    nc.sync.dma_start(out=stab_sb, in_=stab)
    seg2_sb = const.tile([2, B * S], F16, tag="seg2_sb")
    nc.sync.dma_start(out=seg2_sb, in_=seg2.rearrange("b r s -> r b s"))

    t0f = const.tile([1, 128], F32, tag="t0f")
    t1f = const.tile([1, 128], F32, tag="t1f")
    nc.sync.dma_start(out=t0f, in_=stabf[0:1])
    nc.sync.dma_start(out=t1f, in_=stabf[1:2])
    bqs_sb = const.tile([1, 128], F32, tag="bqs_sb")
    nc.sync.dma_start(out=bqs_sb, in_=bqs)
    bqc_sb = const.tile([128, 1], F32, tag="bqc_sb")
    nc.sync.dma_start(out=bqc_sb, in_=bqc)
    bvc_sb = const.tile([128, 1], F32, tag="bvc_sb")
    nc.sync.dma_start(out=bvc_sb, in_=bvc)
    segc_sb = const.tile([128, B * NJT], F32, tag="segc_sb")
    nc.sync.dma_start(out=segc_sb, in_=segc.rearrange("b p t -> p b t"))

    ident = const.tile([128, 128], F16, tag="ident")
    make_identity(nc, ident)
    ones64 = const.tile([1, 64], F16, tag="ones64")
    nc.vector.memset(ones64, 1.0)

    # --- r1 (b_q_s . seg_rep) per-partition bias columns -------------------
    # gamma0_h = sum_{c in head h} bqs[c]*t0[c]; gamma1_h likewise with t1.
    # r1[j] = gamma0_h + (gamma1_h - gamma0_h) * s_j
    prod = const.tile([1, 128], F32, tag="prod")
    g_row = const.tile([1, 4], F32, tag="g_row")   # [g0_h0, g0_h1, g1_h0, g1_h1]
    b_row = const.tile([1, 4], F32, tag="b_row")   # [d_h0, d_h1, g0_h0, g0_h1]
    ones1 = const.tile([1, 128], F32, tag="ones1")
    nc.vector.memset(ones1, 1.0)
    nc.vector.tensor_mul(prod, bqs_sb, t0f)
    nc.vector.tensor_reduce(g_row[0:1, 0:1], prod[0:1, 0:64],
                            axis=mybir.AxisListType.X, op=mybir.AluOpType.add)
    nc.vector.tensor_reduce(g_row[0:1, 1:2], prod[0:1, 64:128],
                            axis=mybir.AxisListType.X, op=mybir.AluOpType.add)
    nc.vector.tensor_mul(prod, bqs_sb, t1f)
    nc.vector.tensor_reduce(g_row[0:1, 2:3], prod[0:1, 0:64],
                            axis=mybir.AxisListType.X, op=mybir.AluOpType.add)
    nc.vector.tensor_reduce(g_row[0:1, 3:4], prod[0:1, 64:128],
                            axis=mybir.AxisListType.X, op=mybir.AluOpType.add)
    # delta_h = g1_h - g0_h in slots 0:2, g0_h in slots 2:4
    nc.vector.tensor_sub(b_row[0:1, 0:2], g_row[0:1, 2:4], g_row[0:1, 0:2])
    nc.vector.tensor_copy(b_row[0:1, 2:4], g_row[0:1, 0:2])
    # broadcast the 4 values to all 128 partitions via a K=1 matmul
    psB = pspool.tile([128, 4], F32, tag="ps_s")
    nc.tensor.matmul(psB, lhsT=ones1, rhs=b_row, start=True, stop=True)
    bc4 = const.tile([128, 4], F32, tag="bc4")
    nc.vector.tensor_copy(bc4, psB)
    # r1 columns, [128, (b*2+hl)*16 + jt]
    r1c = const.tile([128, B * HPC * NJT], F32, tag="r1c")
    for b in range(B):
        for hl in range(HPC):
            nc.vector.tensor_scalar(
                out=r1c[:, (b * HPC + hl) * NJT:(b * HPC + hl + 1) * NJT],
                in0=segc_sb[:, b * NJT:(b + 1) * NJT],
                scalar1=bc4[:, hl:hl + 1],
                scalar2=bc4[:, 2 + hl:2 + hl + 1],
                op0=mybir.AluOpType.mult,
                op1=mybir.AluOpType.add,
            )

    # --- Stage A: projections -> qT, k'T, v_nat ---------------------------
    qT, kT, vn = [None] * B, [None] * B, [None] * B
    if opts.get("no_proj"):
        static_qk = const.tile([128, S], F16, tag="static_qk")
        nc.vector.memset(static_qk, 0.01)
        static_vn = const.tile([128, NJT, DH + 4], F16, tag="static_vn")
        nc.vector.memset(static_vn, 0.01)
        for _b in range(B):
            qT[_b] = static_qk
            kT[_b] = static_qk
            vn[_b] = [static_vn, static_vn]

    def emit_proj(b):
        if opts.get("no_proj"):
            return
        hsb = hspool.tile([128, NKC, S], F16, tag="hsb", name=f"hsb{b}")
        for kk in range(NKC):
            nc.sync.dma_start(out=hsb[:, kk], in_=hsT[b, kk])
        qT_b = qpool.tile([128, S], F16, tag="qT", name=f"qT{b}")
        kT_b = kpool.tile([128, S], F16, tag="kT", name=f"kT{b}")
        vTt = vtpool.tile([128, S], F16, tag="vTt", name=f"vTt{b}")
        for pt in range(NPT if not opts.get("no_proj") else 0):
            sl = bass.ds(pt * 512, 512)
            psQ = pspool.tile([128, 512], F32, tag="ps_s", name="psQ")
            for kk in range(NKC):
                nc.tensor.matmul(psQ, lhsT=w_sb[:, 0, kk], rhs=hsb[:, kk, sl],
                                 start=(kk == 0), stop=(kk == NKC - 1))
            if opts.get("qv_dve"):
                nc.vector.tensor_scalar_add(qT_b[:, sl], psQ, bqc_sb)
            else:
                nc.scalar.add(qT_b[:, sl], psQ, bqc_sb)
            psK = pspool.tile([128, 512], F32, tag="ps_s", name="psK")
            for kk in range(NKC):
                nc.tensor.matmul(psK, lhsT=w_sb[:, 1, kk], rhs=hsb[:, kk, sl],
                                 start=(kk == 0), stop=False)
            nc.tensor.matmul(psK, lhsT=stab_sb,
                             rhs=seg2_sb[:, bass.ds(b * S + pt * 512, 512)],
                             start=False, stop=True)
            nc.vector.tensor_copy(kT_b[:, sl], psK)
            psV = pspool.tile([128, 512], F32, tag="ps_s", name="psV")
            for kk in range(NKC):
                nc.tensor.matmul(psV, lhsT=w_sb[:, 2, kk], rhs=hsb[:, kk, sl],
                                 start=(kk == 0), stop=(kk == NKC - 1))
            if opts.get("qv_dve"):
                nc.vector.tensor_scalar_add(vTt[:, sl], psV, bvc_sb)
            else:
                nc.scalar.add(vTt[:, sl], psV, bvc_sb)
        # v_nat per head: [j, (v cols 0:64 | ones 64:68)] via PE transpose
        vn_b = [vnpool.tile([128, NJT, DH + 4], F16, tag=f"vn{hl}",
                            name=f"vn_b{b}h{hl}") for hl in range(HPC)]
        for jt in range(NJT if not opts.get("no_proj") else 0):
            pst = pspool.tile([128, 128], F16, tag="ps_s", name="pst")
            nc.tensor.transpose(pst, vTt[:, bass.ds(jt * 128, 128)], ident)
            for hl in range(HPC):
                nc.vector.tensor_copy(vn_b[hl][:, jt, bass.ds(0, DH)],
                                      pst[:, bass.ds(hl * DH, DH)])
                nc.gpsimd.memset(vn_b[hl][:, jt, bass.ds(DH, 4)], 1.0)
        qT[b], kT[b], vn[b] = qT_b, kT_b, vn_b

    # --- Stage B ----------------------------------------------------------
    rel = {}
    relx = {}
    static_prob = None
    if opts.get("no_prob"):
        static_prob = const.tile([128, 1024], F16, tag="static_prob")
        nc.vector.memset(static_prob, 0.001)
    static_score = None
    if opts.get("no_qk"):
        static_score = const.tile([128, 1024], F32, tag="static_score")
        nc.vector.memset(static_score, 0.01)

    def emit_rel(ib):
        ibs = bass.ds(ib * 1024, 1024)
        n_fact = len([j for j in range(NJT) if j in factorize_jts])
        if use_mask:
            n_fact = 0
        relb = 2 * (NJT - n_fact) + 2
        relxb = 2 * n_fact + 2
        for jt in range(NJT):
            fact = (jt in factorize_jts) and not use_mask
            for hl in range(HPC):
                tag = "relf" if fact else "rel"
                r = relpool.tile([128, 1024], F16, tag=tag, name=tag,
                                 bufs=(3 if fact else relb))
                nc.sync.dma_start(out=r, in_=relT[hl, bass.ds(jt * 128, 128), ibs])
                if fact:
                    rx = relpool.tile([128, 1024], F16, tag="relx", name="relx",
                                      bufs=relxb)
                    nc.scalar.activation(rx, r, mybir.ActivationFunctionType.Exp)
                    relx[jt, hl] = rx
                else:
                    rel[jt, hl] = r

    def emit_attn(ib, b):
        pv = [pvpool.tile([DH + 4, 1024], F32, tag="pv", name=f"pv{_hl}")
              for _hl in range(HPC)]
        ibs = bass.ds(ib * 1024, 1024)
        for jt in range(NJT):
            if use_mask:
                msk = addpool.tile([128, 1024], F16, tag="msk")
                nc.sync.dma_start(
                    out=msk, in_=maskT[b, bass.ds(jt * 128, 128), ibs])
            inject = (jt in inject_jts) and not use_mask
            fact = (jt in factorize_jts) and not use_mask
            if opts.get("no_qk"):
                psS_all = [static_score, static_score]
            else:
                psS_all = [pspool.tile([128, 1024], F32, tag="ps_s",
                                       name=f"psS{_hl}") for _hl in range(HPC)]
            for i2 in range(2 if not opts.get("no_qk") else 0):
                osl = bass.ds(i2 * 512, 512)
                qsl = bass.ds(ib * 1024 + i2 * 512, 512)
                if inject:
                    for hl in range(HPC):
                        nc.tensor.matmul(psS_all[hl][:, osl], lhsT=ident,
                                         rhs=rel[jt, hl][:, osl],
                                         start=True, stop=False)
                # the two heads' K=64 matmuls are adjacent -> PE row-group
                # packing runs them concurrently (base partitions 0 and 64)
                for hl in range(HPC):
                    hs_ = bass.ds(hl * DH, DH)
                    nc.tensor.matmul(psS_all[hl][:, osl],
                                     lhsT=kT[b][hs_, bass.ds(jt * 128, 128)],
                                     rhs=qT[b][hs_, qsl],
                                     start=not inject, stop=True)
            for hl in range(HPC):
                hs_ = bass.ds(hl * DH, DH)
                psS = psS_all[hl]
                col = (b * HPC + hl) * NJT + jt
                bias_ap = r1c[:, col:col + 1]
                if opts.get("no_prob"):
                    prob = static_prob
                    if False:
                        pass
                    for i2 in range(2 if not opts.get("no_pv") else 0):
                        nc.tensor.matmul(
                            pv[hl][:, bass.ds(i2 * 512, 512)],
                            lhsT=vn[b][hl][:, jt, :],
                            rhs=prob[:, bass.ds(i2 * 512, 512)],
                            start=(jt == 0), stop=(jt == NJT - 1))
                    continue
                prob = probpool.tile([128, 1024], F16, tag="prob")
                if inject:
                    nc.scalar.activation(prob, psS,
                                         mybir.ActivationFunctionType.Exp,
                                         bias=bias_ap, scale=1.0)
                elif fact:
                    eqk = probpool.tile([128, 1024], F16, tag="eqk")
                    nc.scalar.activation(eqk, psS,
                                         mybir.ActivationFunctionType.Exp,
                                         bias=bias_ap, scale=1.0)
                    nc.vector.tensor_mul(prob, eqk, relx[jt, hl])
                else:
                    padd = addpool.tile([128, 1024], F16, tag="padd")
                    nc.vector.tensor_add(padd, psS, rel[jt, hl])
                    if use_mask:
                        padd2 = addpool.tile([128, 1024], F16, tag="padd2")
                        nc.vector.tensor_add(padd2, padd, msk)
                        padd = padd2
                    nc.scalar.activation(prob, padd,
                                         mybir.ActivationFunctionType.Exp,
                                         bias=bias_ap, scale=1.0)
                for i2 in range(2 if not opts.get("no_pv") else 0):
                    nc.tensor.matmul(
                        pv[hl][:, bass.ds(i2 * 512, 512)],
                        lhsT=vn[b][hl][:, jt, :],
                        rhs=prob[:, bass.ds(i2 * 512, 512)],
                        start=(jt == 0), stop=(jt == NJT - 1))
        return pv

    def emit_fin(ib, b, pv):
        if opts.get("no_fin"):
            return
        ibs = bass.ds(ib * 1024, 1024)
        for hl in range(HPC):
            # evacuate PSUM accumulator promptly so the slot frees up
            pvs = ctxpool.tile([DH + 1, 1024], F32, tag="pvs", name="pvs")
            nc.scalar.copy(pvs, pv[hl][0:DH + 1, :])
            rcp1 = denpool.tile([1, 1024], F16, tag="rcp1", name="rcp1")
            with nc.allow_low_precision(reason="1/denom in fp16 is plenty"):
                nc.vector.reciprocal(rcp1, pvs[DH:DH + 1, :])
            psR = pspool.tile([DH, 1024], F32, tag="ps_s", name="psR")
            for i2 in range(2):
                osl = bass.ds(i2 * 512, 512)
                nc.tensor.matmul(psR[:, osl], lhsT=ones64, rhs=rcp1[:, osl],
                                 start=True, stop=True)
            ctxt = ctxpool.tile([DH, 1024], F16, tag="ctxt")
            nc.vector.tensor_mul(ctxt, pvs[0:DH, :], psR)
            nc.sync.dma_start(
                out=out[b, bass.ds(hl * DH, DH), ibs], in_=ctxt)

    # emission order interleaves batch-1 projections under stage B(ib0, b0)
    emit_proj(0)
    emit_rel(0)
    pv00 = emit_attn(0, 0)
    emit_proj(1)
    emit_fin(0, 0, pv00)
    pv01 = emit_attn(0, 1)
    emit_fin(0, 1, pv01)
    emit_rel(1)
    pv10 = emit_attn(1, 0)
    emit_fin(1, 0, pv10)
    pv11 = emit_attn(1, 1)
    emit_fin(1, 1, pv11)


def build_nc(use_mask=False, n_reps=1, inject_jts=INJECT_JTS,
             factorize_jts=None, opts=None):
    nc = bacc.Bacc("TRN2", target_bir_lowering=False, debug=False,
                   num_devices=N_CORES)
    hsT = nc.declare_dram_parameter("hsT", [B, NKC, 128, S], F16, isOutput=False).ap()
    wT = nc.declare_dram_parameter("wT", [3, NKC, 128, 128], F16, isOutput=False).ap()
    relT = nc.declare_dram_parameter("relT", [HPC, S, S], F16, isOutput=False).ap()
    seg2 = nc.declare_dram_parameter("seg2", [B, 2, S], F16, isOutput=False).ap()
    segc = nc.declare_dram_parameter("segc", [B, 128, NJT], F32, isOutput=False).ap()
    stab = nc.declare_dram_parameter("stab", [2, 128], F16, isOutput=False).ap()
    stabf = nc.declare_dram_parameter("stabf", [2, 128], F32, isOutput=False).ap()
    bqs = nc.declare_dram_parameter("bqs", [1, 128], F32, isOutput=False).ap()
    bqc = nc.declare_dram_parameter("bqc", [128, 1], F32, isOutput=False).ap()
    bvc = nc.declare_dram_parameter("bvc", [128, 1], F32, isOutput=False).ap()
    maskT = (nc.declare_dram_parameter("maskT", [B, S, S], F16, isOutput=False).ap()
             if use_mask else None)
    out = nc.declare_dram_parameter("out", [B, 128, S], F16, isOutput=True).ap()
    aps = (hsT, wT, relT, seg2, segc, stab, stabf, bqs, bqc, bvc, maskT, out)

    with tile.TileContext(nc) as tc, ExitStack() as ctx:
        pools = (
            ctx.enter_context(tc.tile_pool(name="const", bufs=1)),
            ctx.enter_context(tc.tile_pool(name="hspool", bufs=1)),
            ctx.enter_context(tc.tile_pool(name="qpool", bufs=B)),
            ctx.enter_context(tc.tile_pool(name="kpool", bufs=B)),
            ctx.enter_context(tc.tile_pool(name="vtpool", bufs=2)),
            ctx.enter_context(tc.tile_pool(name="vnpool", bufs=B)),
            ctx.enter_context(tc.tile_pool(name="relpool", bufs=3)),
            ctx.enter_context(tc.tile_pool(name="addpool", bufs=3)),
            ctx.enter_context(tc.tile_pool(name="probpool", bufs=4)),
            ctx.enter_context(tc.tile_pool(name="pspool", bufs=2, space="PSUM")),
            ctx.enter_context(tc.tile_pool(name="pvpool", bufs=2, space="PSUM")),
            ctx.enter_context(tc.tile_pool(name="denpool", bufs=2)),
            ctx.enter_context(tc.tile_pool(name="rcpbpool", bufs=2)),
            ctx.enter_context(tc.tile_pool(name="ctxpool", bufs=2)),
            ctx.enter_context(tc.tile_pool(name="scrpool", bufs=2, space="DRAM")),
        )
        if factorize_jts is None:
            factorize_jts = FACTORIZE_JTS
        if n_reps == 1:
            emit_body(nc, tc, ctx, pools, aps, use_mask, inject_jts,
                      factorize_jts, opts)
        else:
            hint = (mybir.EngineType.PE, mybir.EngineType.DVE,
                    mybir.EngineType.Activation, mybir.EngineType.SP,
                    mybir.EngineType.Pool)
            with tc.For_i(0, n_reps, 1, hint_engines=hint):
                emit_body(nc, tc, ctx, pools, aps, use_mask, inject_jts,
                          factorize_jts, opts)
    nc.compile()
    return nc


# ---------------------------------------------------------------------------
# host side
# ---------------------------------------------------------------------------

def prep_in_maps(hidden_states, attention_mask, rel_pos, seg_ids,
                 Wq, bq, Wk, Wv, bv, seg_table, b_q_s, use_mask):
    hs = np.asarray(hidden_states, np.float32)
    hsT = np.ascontiguousarray(hs.transpose(0, 2, 1)).astype(np.float16)
    hsT = hsT.reshape(B, NKC, 128, S)
    seg = np.asarray(seg_ids).astype(np.float32)          # [B, S]
    seg2 = np.stack([1.0 - seg, seg], axis=1).astype(np.float16)  # [B,2,S]
    segc = np.ascontiguousarray(
        seg.reshape(B, NJT, 128).transpose(0, 2, 1)).astype(np.float32)
    rel = np.asarray(rel_pos, np.float32)[0]              # [H, S, S]
    relT_all = np.ascontiguousarray(rel.transpose(0, 2, 1)).astype(np.float16)
    Wq = np.asarray(Wq, np.float32)
    Wk = np.asarray(Wk, np.float32) * SCALE               # fold softmax scale
    Wv = np.asarray(Wv, np.float32)
    seg_table = np.asarray(seg_table, np.float32)
    b_q_s = np.asarray(b_q_s, np.float32)
    bq = np.asarray(bq, np.float32); bv = np.asarray(bv, np.float32)
    if use_mask:
        maskT_all = np.ascontiguousarray(
            np.asarray(attention_mask, np.float32)[:, 0].transpose(0, 2, 1)
        ).astype(np.float16)

    in_maps = []
    for c in range(N_CORES):
        hc = slice(c * HPC * DH, (c + 1) * HPC * DH)      # 128 head-columns
        wT = np.stack([
            np.ascontiguousarray(Wq[hc].T),
            np.ascontiguousarray(Wk[hc].T),
            np.ascontiguousarray(Wv[hc].T),
        ]).astype(np.float16).reshape(3, NKC, 128, 128)
        m = {
            "hsT": hsT,
            "wT": wT,
            "relT": relT_all[c * HPC:(c + 1) * HPC],
            "seg2": seg2,
            "segc": segc,
            "stab": seg_table[:, hc].astype(np.float16),
            "stabf": seg_table[:, hc].astype(np.float32),
            "bqs": b_q_s[0, c * HPC:(c + 1) * HPC, 0].reshape(1, 128).astype(np.float32),
            "bqc": bq[hc].reshape(128, 1).astype(np.float32),
            "bvc": bv[hc].reshape(128, 1).astype(np.float32),
        }
        if use_mask:
            m["maskT"] = maskT_all
        in_maps.append(m)
    return in_maps


def assemble_output(results):
    out = np.empty((B, S, D), np.float32)
    for c in range(N_CORES):
        ctxT = results[c]["out"].astype(np.float32)       # [B, 128, S]
        hc = slice(c * HPC * DH, (c + 1) * HPC * DH)
        out[:, :, hc] = ctxT.transpose(0, 2, 1)
    return out


_CACHED = {}


def kernel(**inputs):
    use_mask = bool(np.any(np.asarray(inputs["attention_mask"])))
    key = ("nc", use_mask)
    if key not in _CACHED:
        _CACHED[key] = build_nc(use_mask=use_mask)
    nc = _CACHED[key]
    in_maps = prep_in_maps(use_mask=use_mask, **inputs)
    res = run_bass_kernel_spmd(nc, in_maps, list(range(N_CORES)))
    return assemble_output(res.results)

